# revision 1
# baseline (speedup 1.0000x reference)
"""EPLL MoE-routing kernel for 8 trn2 NeuronCores.

Strategy (data-parallel over patches, per sharding hint):
- Host precomputes per-beta GMM params (A = Sigma_reg^-1 packed symmetric,
  Amu, const terms) and the patch outer-products, padded+sharded 8 ways.
- Device (Bass, SPMD on cores 0-7): per core, the dense routing matmul
  logpost[k, p] = sum_r Aaug[r, k] * OTaug[r, p]  (r = 703 rows: 666
  sym-packed outer entries, 36 patch entries for the cross term, 1 ones
  row for the constant) as 6x128-row bf16 accumulating matmuls (fp32
  PSUM accumulate) per 512-patch block.
- Host: argmax over k, Wiener apply est = E[k*] x_p, overlap-add, blend.

Self-contained: shapes hardcoded for y[1,1,256,256], K=200, D=36.
"""

import sys

sys.path.insert(0, "/opt/trn_rl_repo")

import numpy as np
import ml_dtypes

B, C, H, W = 1, 1, 256, 256
PS = 6
K = 200
D = PS * PS * C            # 36
SIGMA_SQ = 0.01
BETAS = [b / SIGMA_SQ for b in (1.0, 4.0, 8.0, 16.0, 32.0)]
NPIX = C * H * W

NI = H - PS + 1            # 251
P = NI * NI                # 63001
N_CORES = 8
PPAD = 65536               # padded patch count
PPC = PPAD // N_CORES      # 8192 patches per core
NBLK = PPC // 512          # 16 blocks of 512 patches
NSYM = D * (D + 1) // 2    # 666
NROW = NSYM + D + 1        # 703 rows of the augmented operand
NROW_PAD = 704             # -> 5 chunks of 128 + 1 of 64
NCHUNK = 6

_IU, _IV = np.triu_indices(D)          # sym packing order (d <= e)
_SYM_SCALE = np.where(_IU == _IV, 1.0, 2.0).astype(np.float32)


def _patch_linear_indices():
    i0 = np.arange(NI)
    rows = i0[:, None, None, None] + np.arange(PS)[None, None, :, None]
    cols = i0[None, :, None, None] + np.arange(PS)[None, None, None, :]
    lin = (rows * W + cols).reshape(NI * NI, PS * PS)
    return lin.astype(np.int64)


LIN = _patch_linear_indices()          # [P, D]

_STATE = {}


def _build_bass():
    from concourse import bacc, mybir
    from concourse.tile import TileContext

    nc = bacc.Bacc("TRN2", target_bir_lowering=False, debug=False,
                   num_devices=N_CORES)

    ot_dram = nc.dram_tensor("ot", [128, 5, PPC], mybir.dt.bfloat16,
                             kind="ExternalInput")
    ott_dram = nc.dram_tensor("ott", [64, PPC], mybir.dt.bfloat16,
                              kind="ExternalInput")
    a_dram = nc.dram_tensor("a", [128, 5, K], mybir.dt.bfloat16,
                            kind="ExternalInput")
    at_dram = nc.dram_tensor("at", [64, K], mybir.dt.bfloat16,
                             kind="ExternalInput")
    lp_dram = nc.dram_tensor("lp", [2, NBLK, 100, 512], mybir.dt.float32,
                             kind="ExternalOutput")

    f32r = mybir.dt.float32r

    with TileContext(nc) as tc:
        with (
            tc.tile_pool(name="apool", bufs=1) as apool,
            tc.tile_pool(name="otpool", bufs=5) as otpool,
            tc.tile_pool(name="lppool", bufs=5) as lppool,
            tc.tile_pool(name="psum", bufs=8, space="PSUM") as pspool,
        ):
            a_sb = apool.tile([128, 5, K], mybir.dt.bfloat16)
            nc.gpsimd.dma_start(a_sb[:], a_dram.ap())
            at_sb = apool.tile([128, K], mybir.dt.bfloat16, tag="at")
            nc.gpsimd.dma_start(at_sb[0:64, :], at_dram.ap())

            for b in range(NBLK):
                ot = otpool.tile([128, 5, 512], mybir.dt.bfloat16,
                                 tag="ot")
                ott = otpool.tile([128, 512], mybir.dt.bfloat16, tag="ott")
                nc.sync.dma_start(
                    ot[:, 0:3, :],
                    ot_dram.ap()[:, 0:3, b * 512:(b + 1) * 512])
                nc.gpsimd.dma_start(
                    ot[:, 3:5, :],
                    ot_dram.ap()[:, 3:5, b * 512:(b + 1) * 512])
                nc.gpsimd.dma_start(
                    ott[0:64, :], ott_dram.ap()[:, b * 512:(b + 1) * 512])
                for kh in range(2):
                    ps = pspool.tile([128, 512], mybir.dt.float32, tag="lp")
                    for c in range(5):
                        nc.tensor.matmul(
                            ps[0:100, :],
                            a_sb[:, c, kh * 100:(kh + 1) * 100],
                            ot[:, c, :],
                            start=(c == 0), stop=False)
                    nc.tensor.matmul(
                        ps[0:100, :],
                        at_sb[0:64, kh * 100:(kh + 1) * 100],
                        ott[0:64, :],
                        start=False, stop=True)
                    lp_sb = lppool.tile([128, 512], mybir.dt.float32,
                                        tag="lpsb")
                    nc.scalar.copy(lp_sb[0:100, :], ps[0:100, :])
                    nc.sync.dma_start(lp_dram.ap()[kh, b], lp_sb[0:100, :])
    nc.finalize()
    return nc


def _get_state():
    if not _STATE:
        _STATE["nc"] = _build_bass()
    return _STATE


def kernel(y, mu, log_weights, eigvecs, eigvals):
    from concourse import bass_utils

    y = np.asarray(y, np.float32)
    mu = np.asarray(mu, np.float32)
    lw = np.asarray(log_weights, np.float32)
    U = np.asarray(eigvecs, np.float32)
    ev = np.asarray(eigvals, np.float32)

    st = _get_state()
    nc = st["nc"]

    yf = y.reshape(-1)
    x = yf.copy()

    mult = np.bincount(LIN.ravel(), minlength=NPIX).astype(np.float32)
    inv_mult = 1.0 / mult

    for beta in BETAS:
        reg = 1.0 / beta
        l = ev + reg                                        # [K, D]
        il = (1.0 / l).astype(np.float32)
        A = np.einsum("kde,ke,kfe->kdf", U, il, U)          # [K, D, D]
        E = np.einsum("kde,ke,kfe->kdf", U, ev * il, U)     # [K, D, D]
        logdet = np.log(l).sum(1)
        Amu = np.einsum("kdf,kf->kd", A, mu)                # [K, D]
        muAmu = np.einsum("kd,kd->k", mu, Amu)
        cterm = (lw - 0.5 * logdet - 0.5 * muAmu).astype(np.float32)

        # augmented stationary operand [NROW_PAD, K]
        Aaug = np.zeros((NROW_PAD, K), np.float32)
        Aaug[:NSYM] = (-0.5 * _SYM_SCALE[:, None]
                       * A[:, _IU, _IV].T.astype(np.float32))
        Aaug[NSYM:NSYM + D] = Amu.T
        Aaug[NSYM + D] = cterm
        Ab = Aaug.astype(ml_dtypes.bfloat16)
        a_in = np.ascontiguousarray(
            Ab[:640].reshape(5, 128, K).transpose(1, 0, 2))
        at_in = np.ascontiguousarray(Ab[640:704])

        # augmented moving operand [NROW_PAD, PPAD]
        pat = x[LIN]                                        # [P, D]
        OT = np.zeros((NROW_PAD, PPAD), np.float32)
        OT[:NSYM, :P] = (pat[:, _IU] * pat[:, _IV]).T
        OT[NSYM:NSYM + D, :P] = pat.T
        OT[NSYM + D, :P] = 1.0
        OTb = OT.astype(ml_dtypes.bfloat16)

        in_maps = []
        for c in range(N_CORES):
            sl = OTb[:, c * PPC:(c + 1) * PPC]
            otc = np.ascontiguousarray(
                sl[:640].reshape(5, 128, PPC).transpose(1, 0, 2))
            ottc = np.ascontiguousarray(sl[640:704])
            in_maps.append({"ot": otc, "ott": ottc,
                            "a": a_in, "at": at_in})

        res = bass_utils.run_bass_kernel_spmd(
            nc, in_maps, core_ids=list(range(N_CORES)))

        lp = np.concatenate(
            [r["lp"].reshape(2, NBLK, 100, 512).transpose(0, 2, 1, 3)
             .reshape(K, PPC) for r in res.results], axis=1)   # [K, PPAD]
        ks = lp[:, :P].argmax(0)                             # [P]

        est = np.einsum("pde,pe->pd", E[ks], pat)
        xt = np.bincount(LIN.ravel(), weights=est.ravel().astype(np.float64),
                         minlength=NPIX).astype(np.float32)
        xt *= inv_mult
        cdf = beta * SIGMA_SQ
        x = (yf + cdf * xt) / (1.0 + cdf)

    return x.reshape(B, C, H, W).astype(np.float32)



# revision 2
# speedup vs baseline: 1.2093x; 1.2093x over previous
"""EPLL MoE-routing kernel for 8 trn2 NeuronCores — fp8 DoubleRow, transposed.

Device (per core, per beta): routing matmul producing lp[p, k]
    lp[p, k] = sum_r OTaug[r, p] * Aaug[r, k]
with r = 702 rows (666 sym-packed outer products of centered patches +
36 linear rows; constant term added on host), K = 200 (padded to 208
moving cols), patches sharded 8 ways (7936/core, 62 groups of 128).
Patch-product chunks are the STATIONARY operand ([h, 2, 128] per
128-patch group), the A operand is MOVING ([h, 2, 208]) — so each
DoubleRow matmul costs 104 PE cycles and the psum output [128p, 208k]
has patches on partitions.  fp8 e4m3 operands, fp32 PSUM accumulate,
fp16 eviction (Act/DVE copies), [128, 62, 200] f16 output.

Self-contained: shapes hardcoded for y[1,1,256,256], K=200, D=36.
"""

import sys

sys.path.insert(0, "/opt/trn_rl_repo")

import numpy as np
import ml_dtypes

B, C, H, W = 1, 1, 256, 256
PS = 6
K = 200
KPAD = 208                 # moving cols padded to 16 for DoubleRow stride
D = PS * PS * C            # 36
SIGMA_SQ = 0.01
BETAS = [b / SIGMA_SQ for b in (1.0, 4.0, 8.0, 16.0, 32.0)]
NPIX = C * H * W

NI = H - PS + 1            # 251
P = NI * NI                # 63001
N_CORES = 8
PPC = 7936                 # padded patches per core (8*7936 = 63488)
NPG = PPC // 128           # 62 patch groups of 128

NSYM = D * (D + 1) // 2    # 666
NROW = NSYM + D            # 702
CH = [128, 128, 96]        # contraction chunk heights (x2 rows each)
CBASE = [0, 256, 512]

CENTER = 0.5
SQ = 16.0                  # product-row scale (A side divided by SQ)
SL = 8.0                   # linear-row scale

E4 = ml_dtypes.float8_e4m3fn

_IU, _IV = np.triu_indices(D)
_SYM_SCALE = np.where(_IU == _IV, 1.0, 2.0).astype(np.float32)


def _patch_linear_indices():
    i0 = np.arange(NI)
    rows = i0[:, None, None, None] + np.arange(PS)[None, None, :, None]
    cols = i0[None, :, None, None] + np.arange(PS)[None, None, None, :]
    return (rows * W + cols).reshape(NI * NI, PS * PS).astype(np.int64)


LIN = _patch_linear_indices()          # [P, D]

_STATE = {}


# schedule: input groups (contiguous dram tensors, sizes in patch cols,
# multiples of 512) with queue assignment; output groups (sizes in patch
# groups of 128); copy engine per output unit (0=Act, 2=DVE).
# queues: "sp" (SP hwdge), "act" (Activation hwdge), "pool" (SWDGE)
IN_GRP = [(128, "pool"), (512, "sp"), (512, "act"), (1024, "pool"),
          (1024, "sp"), (1024, "pool"), (1024, "sp"), (1408, "pool"),
          (1280, "sp")]
# 62 patch groups -> copy units of 4 groups (last 2): 16 units
OUT_UNITS = [4] * 15 + [2]
# out-DMA groups: number of copy units per DMA and queue
OUT_DMA = [(3, "act"), (3, "sp"), (3, "act"), (3, "pool"), (2, "sp"),
           (1, "pool"), (1, "act")]
COPY_PAT = [2, 2, 2, 0, 2, 2, 2, 0, 2, 2, 0, 2, 2, 2, 0, 0]


def _build_bass():
    from concourse import bacc, mybir
    from concourse.tile import TileContext

    nc = bacc.Bacc("TRN2", target_bir_lowering=False, debug=False,
                   num_devices=N_CORES)

    in_cols = [g[0] for g in IN_GRP]
    assert sum(in_cols) == PPC
    assert sum(OUT_UNITS) == NPG
    assert sum(n for n, _ in OUT_DMA) == len(OUT_UNITS)
    in_edges = np.cumsum([0] + in_cols).tolist()

    ot_drams = [
        nc.dram_tensor(f"ot{g}", [128, 3, 2, gc], mybir.dt.float8e4,
                       kind="ExternalInput")
        for g, (gc, _) in enumerate(IN_GRP)]
    a_dram = nc.dram_tensor("a", [128, 3, 2, KPAD], mybir.dt.float8e4,
                            kind="ExternalInput")
    lp_dram = nc.dram_tensor("lp", [128, NPG, K], mybir.dt.float16,
                             kind="ExternalOutput")

    DR = mybir.MatmulPerfMode.DoubleRow

    with TileContext(nc) as tc:
        qmap = {"sp": nc.sync, "act": nc.scalar, "pool": nc.gpsimd}
        with (
            tc.tile_pool(name="apool", bufs=1) as apool,
            tc.tile_pool(name="otpool", bufs=1) as otpool,
            tc.tile_pool(name="lppool", bufs=1) as lppool,
            tc.tile_pool(name="psum", bufs=4, space="PSUM") as pspool,
        ):
            a_sb = apool.tile([128, 3, 2, KPAD], mybir.dt.float8e4)
            nc.sync.dma_start(a_sb[:], a_dram.ap())

            in_tiles = []
            for g, (gcols, q) in enumerate(IN_GRP):
                ot = otpool.tile([128, 3, 2, gcols], mybir.dt.float8e4,
                                 tag=f"ot{g}")
                qmap[q].dma_start(ot[:], ot_drams[g].ap())
                in_tiles.append((ot, in_edges[g], in_edges[g + 1]))

            # output-unit loop: each unit = up to 4 patch groups of 128,
            # copy units accumulate into one lp tile per OUT_DMA group
            pg = 0                      # global patch-group index
            dma_i = 0                   # index into OUT_DMA
            dma_fill = 0                # units accumulated toward OUT_DMA
            lp_sb = None
            for u, ng in enumerate(OUT_UNITS):
                ps = pspool.tile([128, 4, 256], mybir.dt.float32, tag="ps")
                for j in range(ng):
                    c0 = (pg + j) * 128
                    for ot, g0, g1 in in_tiles:
                        if g0 <= c0 < g1:
                            break
                    off = c0 - g0
                    for c in range(3):
                        h = CH[c]
                        nc.tensor.matmul(
                            ps[:, j, 0:KPAD],
                            ot[0:h, c, :, off:off + 128],
                            a_sb[0:h, c],
                            start=(c == 0), stop=(c == 2),
                            perf_mode=DR)
                if lp_sb is None:
                    ng_dma = sum(
                        OUT_UNITS[u + i] for i in range(OUT_DMA[dma_i][0]))
                    lp_sb = lppool.tile([128, ng_dma, K], mybir.dt.float16,
                                        tag=f"lp{dma_i}")
                    lp_pg0 = pg
                lo = pg - lp_pg0
                dst = lp_sb[:, lo:lo + ng, :]
                if COPY_PAT[u] == 0:
                    nc.scalar.copy(dst, ps[:, 0:ng, 0:K])
                else:
                    nc.vector.tensor_copy(dst, ps[:, 0:ng, 0:K])
                pg += ng
                dma_fill += 1

                if dma_fill == OUT_DMA[dma_i][0]:
                    q = qmap[OUT_DMA[dma_i][1]]
                    q.dma_start(lp_dram.ap()[:, lp_pg0:pg, :], lp_sb[:])
                    lp_sb = None
                    dma_fill = 0
                    dma_i += 1
    nc.finalize()
    return nc


def _get_state():
    if not _STATE:
        _STATE["nc"] = _build_bass()
    return _STATE


def _pack_rows(rows_e4):
    """rows_e4: [NROW(702), cols] fp8 -> [128, 3, 2, cols] chunk layout."""
    out = np.zeros((128, 3, 2, rows_e4.shape[1]), E4)
    for c in range(3):
        for i in range(2):
            lo = CBASE[c] + i * CH[c]
            hi = min(lo + CH[c], NROW)
            if lo >= NROW:
                continue
            out[0:hi - lo, c, i] = rows_e4[lo:hi]
    return out


def kernel(y, mu, log_weights, eigvecs, eigvals):
    from concourse import bass_utils

    y = np.asarray(y, np.float32)
    mu = np.asarray(mu, np.float32)
    lw = np.asarray(log_weights, np.float32)
    U = np.asarray(eigvecs, np.float32)
    ev = np.asarray(eigvals, np.float32)

    st = _get_state()
    nc = st["nc"]

    yf = y.reshape(-1)
    x = yf.copy()

    mult = np.bincount(LIN.ravel(), minlength=NPIX).astype(np.float32)
    inv_mult = 1.0 / mult

    in_edges = np.cumsum([0] + [g[0] for g in IN_GRP]).tolist()

    for beta in BETAS:
        reg = 1.0 / beta
        l = ev + reg                                        # [K, D]
        il = (1.0 / l).astype(np.float32)
        A = np.einsum("kde,ke,kfe->kdf", U, il, U)          # [K, D, D]
        E = np.einsum("kde,ke,kfe->kdf", U, ev * il, U)     # [K, D, D]
        logdet = np.log(l).sum(1)
        mu_c = mu - CENTER
        Amu = np.einsum("kdf,kf->kd", A, mu_c)              # [K, D]
        muAmu = np.einsum("kd,kd->k", mu_c, Amu)
        cterm = (lw - 0.5 * logdet - 0.5 * muAmu).astype(np.float32)

        # moving operand [NROW, KPAD] fp8 -> [128, 3, 2, KPAD]
        Arows = np.zeros((NROW, KPAD), np.float32)
        Arows[:NSYM, :K] = (-0.5 / SQ * _SYM_SCALE[:, None]
                            * A[:, _IU, _IV].T)
        Arows[NSYM:, :K] = Amu.T / SL
        a_pack = _pack_rows(Arows.astype(E4))               # [128,3,2,KPAD]

        # stationary operand rows [NROW, P] fp8, shard per core
        pat = x[LIN]                                        # [P, D] f32
        pc = pat - CENTER
        rows = np.empty((P, NROW), np.float32)
        np.multiply(pc[:, _IU], pc[:, _IV], out=rows[:, :NSYM])
        rows[:, :NSYM] *= SQ
        rows[:, NSYM:] = pc * SL
        rows_e4 = rows.astype(E4)                           # [P, NROW]

        in_maps = []
        for cidx in range(N_CORES):
            p0 = cidx * PPC
            p1 = min(p0 + PPC, P)
            slab = np.zeros((NROW, PPC), E4)
            slab[:, 0:p1 - p0] = rows_e4[p0:p1].T
            packed = _pack_rows(slab)                       # [128,3,2,PPC]
            m = {"a": a_pack}
            for g in range(len(IN_GRP)):
                m[f"ot{g}"] = np.ascontiguousarray(
                    packed[:, :, :, in_edges[g]:in_edges[g + 1]])
            in_maps.append(m)

        res = bass_utils.run_bass_kernel_spmd(
            nc, in_maps, core_ids=list(range(N_CORES)))

        # lp [128, NPG, K] per core: patch p = pg*128 + part
        lp_all = np.concatenate(
            [r["lp"].transpose(1, 0, 2).reshape(PPC, K)
             for r in res.results], axis=0)                 # [8*PPC, K]
        lp_full = np.concatenate(
            [lp_all[cidx * PPC: cidx * PPC + min(PPC, P - cidx * PPC)]
             for cidx in range(N_CORES)], axis=0)           # [P, K]
        lp_full = lp_full.astype(np.float32) + cterm[None, :]

        # exact top-candidate repair: device lp ranks candidates; host
        # re-evaluates the top TOPC+1 exactly and picks the true best
        TOPC = 4
        cand = np.argpartition(-lp_full, TOPC, axis=1)[:, :TOPC + 1]
        best_v = None
        best_k = None
        for r in range(TOPC + 1):
            kr = cand[:, r]
            quad = np.einsum("pde,pd,pe->p", A[kr], pc, pc, optimize=True)
            lin = np.einsum("pd,pd->p", Amu[kr], pc)
            v = -0.5 * quad + lin + cterm[kr]
            if best_v is None:
                best_v, best_k = v, kr.copy()
            else:
                m = v > best_v
                best_v = np.where(m, v, best_v)
                best_k = np.where(m, kr, best_k)
        ks = best_k

        est = np.einsum("pde,pe->pd", E[ks], pat)
        xt = np.bincount(LIN.ravel(), weights=est.ravel().astype(np.float64),
                        minlength=NPIX).astype(np.float32)
        xt *= inv_mult
        cdf = beta * SIGMA_SQ
        x = (yf + cdf * xt) / (1.0 + cdf)

    return x.reshape(B, C, H, W).astype(np.float32)


# revision 3
# speedup vs baseline: 1.2370x; 1.0229x over previous
"""EPLL MoE-routing kernel for 8 trn2 NeuronCores — fp8 DoubleRow, transposed.

Device (per core, per beta): routing matmul producing lp[p, k]
    lp[p, k] = sum_r OTaug[r, p] * Aaug[r, k]
with r = 702 rows (666 sym-packed outer products of centered patches +
36 linear rows; constant term added on host), K = 200 (padded to 208
moving cols), patches sharded 8 ways (7936/core, 62 groups of 128).
Patch-product chunks are the STATIONARY operand ([h, 2, 128] per
128-patch group), the A operand is MOVING ([h, 2, 208]) — so each
DoubleRow matmul costs 104 PE cycles and the psum output [128p, 208k]
has patches on partitions.  fp8 e4m3 operands, fp32 PSUM accumulate,
fp16 eviction (Act/DVE copies), [128, 62, 200] f16 output.

Self-contained: shapes hardcoded for y[1,1,256,256], K=200, D=36.
"""

import sys

sys.path.insert(0, "/opt/trn_rl_repo")

import numpy as np
import ml_dtypes

B, C, H, W = 1, 1, 256, 256
PS = 6
K = 200
KPAD = 208                 # moving cols padded to 16 for DoubleRow stride
D = PS * PS * C            # 36
SIGMA_SQ = 0.01
BETAS = [b / SIGMA_SQ for b in (1.0, 4.0, 8.0, 16.0, 32.0)]
NPIX = C * H * W

NI = H - PS + 1            # 251
P = NI * NI                # 63001
N_CORES = 8
PPC = 7936                 # padded patches per core (8*7936 = 63488)
NPG = PPC // 128           # 62 patch groups of 128

NSYM = D * (D + 1) // 2    # 666
NROW = NSYM + D            # 702
CH = [128, 128, 96]        # contraction chunk heights (x2 rows each)
CBASE = [0, 256, 512]

CENTER = 0.5
# operand scales chosen so the device matmul result equals lp/OSCALE
# (folds the fp8-output range scaling into the operands)
SQ = 2.0                   # product-row scale
SL = 2.0                   # linear-row scale
OSCALE = 16.0              # host multiplies fp8 output by this

E4 = ml_dtypes.float8_e4m3fn

_IU, _IV = np.triu_indices(D)
_SYM_SCALE = np.where(_IU == _IV, 1.0, 2.0).astype(np.float32)


def _patch_linear_indices():
    i0 = np.arange(NI)
    rows = i0[:, None, None, None] + np.arange(PS)[None, None, :, None]
    cols = i0[None, :, None, None] + np.arange(PS)[None, None, None, :]
    return (rows * W + cols).reshape(NI * NI, PS * PS).astype(np.int64)


LIN = _patch_linear_indices()          # [P, D]

_STATE = {}


# schedule: input groups (contiguous dram tensors, sizes in patch cols,
# multiples of 512) with queue assignment; output groups (sizes in patch
# groups of 128); copy engine per output unit (0=Act, 2=DVE).
# queues: "sp" (SP hwdge), "act" (Activation hwdge), "pool" (SWDGE)
IN_GRP = [(128, "pool"), (512, "sp"), (512, "act"), (1024, "pool"),
          (1024, "sp"), (1024, "act"), (1024, "pool"), (1408, "sp"),
          (1280, "pool")]
# 62 patch groups -> copy units of 4 groups (last 2): 16 units
OUT_UNITS = [4] * 15 + [2]
# out-DMA groups: number of copy units per DMA and queue
OUT_DMA = [(3, "sp"), (3, "pool"), (3, "sp"), (3, "pool"), (2, "act"),
           (1, "sp"), (1, "act")]
COPY_PAT = [2, 0, 2, 2, 0, 2, 2, 0, 2, 0, 2, 2, 0, 2, 0, 0]


def _build_bass():
    from concourse import bacc, mybir
    from concourse.tile import TileContext

    nc = bacc.Bacc("TRN2", target_bir_lowering=False, debug=False,
                   num_devices=N_CORES)

    in_cols = [g[0] for g in IN_GRP]
    assert sum(in_cols) == PPC
    assert sum(OUT_UNITS) == NPG
    assert sum(n for n, _ in OUT_DMA) == len(OUT_UNITS)
    in_edges = np.cumsum([0] + in_cols).tolist()

    ot_drams = [
        nc.dram_tensor(f"ot{g}", [128, 3, 2, gc], mybir.dt.float8e4,
                       kind="ExternalInput")
        for g, (gc, _) in enumerate(IN_GRP)]
    a_dram = nc.dram_tensor("a", [128, 3, 2, KPAD], mybir.dt.float8e4,
                            kind="ExternalInput")
    lp_dram = nc.dram_tensor("lp", [128, NPG, K], mybir.dt.float8e4,
                             kind="ExternalOutput")

    DR = mybir.MatmulPerfMode.DoubleRow

    with TileContext(nc) as tc:
        qmap = {"sp": nc.sync, "act": nc.scalar, "pool": nc.gpsimd}
        with (
            tc.tile_pool(name="apool", bufs=1) as apool,
            tc.tile_pool(name="otpool", bufs=1) as otpool,
            tc.tile_pool(name="lppool", bufs=1) as lppool,
            tc.tile_pool(name="psum", bufs=4, space="PSUM") as pspool,
        ):
            a_sb = apool.tile([128, 3, 2, KPAD], mybir.dt.float8e4)
            nc.sync.dma_start(a_sb[:], a_dram.ap())

            in_tiles = []
            for g, (gcols, q) in enumerate(IN_GRP):
                ot = otpool.tile([128, 3, 2, gcols], mybir.dt.float8e4,
                                 tag=f"ot{g}")
                qmap[q].dma_start(ot[:], ot_drams[g].ap())
                in_tiles.append((ot, in_edges[g], in_edges[g + 1]))

            # output-unit loop: each unit = up to 4 patch groups of 128,
            # copy units accumulate into one lp tile per OUT_DMA group
            pg = 0                      # global patch-group index
            dma_i = 0                   # index into OUT_DMA
            dma_fill = 0                # units accumulated toward OUT_DMA
            lp_sb = None
            for u, ng in enumerate(OUT_UNITS):
                ps = pspool.tile([128, 4, 256], mybir.dt.float32, tag="ps")
                for j in range(ng):
                    c0 = (pg + j) * 128
                    for ot, g0, g1 in in_tiles:
                        if g0 <= c0 < g1:
                            break
                    off = c0 - g0
                    for c in range(3):
                        h = CH[c]
                        nc.tensor.matmul(
                            ps[:, j, 0:KPAD],
                            ot[0:h, c, :, off:off + 128],
                            a_sb[0:h, c],
                            start=(c == 0), stop=(c == 2),
                            perf_mode=DR)
                if lp_sb is None:
                    ng_dma = sum(
                        OUT_UNITS[u + i] for i in range(OUT_DMA[dma_i][0]))
                    lp_sb = lppool.tile([128, ng_dma, K],
                                        mybir.dt.float8e4,
                                        tag=f"lp{dma_i}")
                    lp_pg0 = pg
                lo = pg - lp_pg0
                dst = lp_sb[:, lo:lo + ng, :]
                if COPY_PAT[u] == 0:
                    nc.scalar.copy(dst, ps[:, 0:ng, 0:K])
                else:
                    nc.vector.tensor_copy(dst, ps[:, 0:ng, 0:K])
                pg += ng
                dma_fill += 1

                if dma_fill == OUT_DMA[dma_i][0]:
                    q = qmap[OUT_DMA[dma_i][1]]
                    q.dma_start(lp_dram.ap()[:, lp_pg0:pg, :], lp_sb[:])
                    lp_sb = None
                    dma_fill = 0
                    dma_i += 1
    nc.finalize()
    return nc


def _get_state():
    if not _STATE:
        _STATE["nc"] = _build_bass()
    return _STATE


def _pack_rows(rows_e4):
    """rows_e4: [NROW(702), cols] fp8 -> [128, 3, 2, cols] chunk layout."""
    out = np.zeros((128, 3, 2, rows_e4.shape[1]), E4)
    for c in range(3):
        for i in range(2):
            lo = CBASE[c] + i * CH[c]
            hi = min(lo + CH[c], NROW)
            if lo >= NROW:
                continue
            out[0:hi - lo, c, i] = rows_e4[lo:hi]
    return out


def kernel(y, mu, log_weights, eigvecs, eigvals):
    from concourse import bass_utils

    y = np.asarray(y, np.float32)
    mu = np.asarray(mu, np.float32)
    lw = np.asarray(log_weights, np.float32)
    U = np.asarray(eigvecs, np.float32)
    ev = np.asarray(eigvals, np.float32)

    st = _get_state()
    nc = st["nc"]

    yf = y.reshape(-1)
    x = yf.copy()

    mult = np.bincount(LIN.ravel(), minlength=NPIX).astype(np.float32)
    inv_mult = 1.0 / mult

    in_edges = np.cumsum([0] + [g[0] for g in IN_GRP]).tolist()

    for beta in BETAS:
        reg = 1.0 / beta
        l = ev + reg                                        # [K, D]
        il = (1.0 / l).astype(np.float32)
        A = np.einsum("kde,ke,kfe->kdf", U, il, U)          # [K, D, D]
        E = np.einsum("kde,ke,kfe->kdf", U, ev * il, U)     # [K, D, D]
        logdet = np.log(l).sum(1)
        mu_c = mu - CENTER
        Amu = np.einsum("kdf,kf->kd", A, mu_c)              # [K, D]
        muAmu = np.einsum("kd,kd->k", mu_c, Amu)
        cterm = (lw - 0.5 * logdet - 0.5 * muAmu).astype(np.float32)

        # moving operand [NROW, KPAD] fp8 -> [128, 3, 2, KPAD]
        Arows = np.zeros((NROW, KPAD), np.float32)
        Arows[:NSYM, :K] = (-0.5 / (SQ * OSCALE) * _SYM_SCALE[:, None]
                            * A[:, _IU, _IV].T)
        Arows[NSYM:, :K] = Amu.T / (SL * OSCALE)
        a_pack = _pack_rows(Arows.astype(E4))               # [128,3,2,KPAD]

        # stationary operand rows [NROW, P] fp8, shard per core
        pat = x[LIN]                                        # [P, D] f32
        pc = pat - CENTER
        rows = np.empty((P, NROW), np.float32)
        np.multiply(pc[:, _IU], pc[:, _IV], out=rows[:, :NSYM])
        rows[:, :NSYM] *= SQ
        rows[:, NSYM:] = pc * SL
        rows_e4 = rows.astype(E4)                           # [P, NROW]

        in_maps = []
        for cidx in range(N_CORES):
            p0 = cidx * PPC
            p1 = min(p0 + PPC, P)
            slab = np.zeros((NROW, PPC), E4)
            slab[:, 0:p1 - p0] = rows_e4[p0:p1].T
            packed = _pack_rows(slab)                       # [128,3,2,PPC]
            m = {"a": a_pack}
            for g in range(len(IN_GRP)):
                m[f"ot{g}"] = np.ascontiguousarray(
                    packed[:, :, :, in_edges[g]:in_edges[g + 1]])
            in_maps.append(m)

        res = bass_utils.run_bass_kernel_spmd(
            nc, in_maps, core_ids=list(range(N_CORES)))

        # lp [128, NPG, K] per core: patch p = pg*128 + part
        lp_all = np.concatenate(
            [r["lp"].transpose(1, 0, 2).reshape(PPC, K)
             for r in res.results], axis=0)                 # [8*PPC, K]
        lp_full = np.concatenate(
            [lp_all[cidx * PPC: cidx * PPC + min(PPC, P - cidx * PPC)]
             for cidx in range(N_CORES)], axis=0)           # [P, K]
        lp_full = lp_full.astype(np.float32) * OSCALE + cterm[None, :]

        # exact top-candidate repair: device lp ranks candidates; host
        # re-evaluates the top TOPC+1 exactly and picks the true best
        TOPC = 4
        cand = np.argpartition(-lp_full, TOPC, axis=1)[:, :TOPC + 1]
        best_v = None
        best_k = None
        for r in range(TOPC + 1):
            kr = cand[:, r]
            quad = np.einsum("pde,pd,pe->p", A[kr], pc, pc, optimize=True)
            lin = np.einsum("pd,pd->p", Amu[kr], pc)
            v = -0.5 * quad + lin + cterm[kr]
            if best_v is None:
                best_v, best_k = v, kr.copy()
            else:
                m = v > best_v
                best_v = np.where(m, v, best_v)
                best_k = np.where(m, kr, best_k)
        ks = best_k

        est = np.einsum("pde,pe->pd", E[ks], pat)
        xt = np.bincount(LIN.ravel(), weights=est.ravel().astype(np.float64),
                        minlength=NPIX).astype(np.float32)
        xt *= inv_mult
        cdf = beta * SIGMA_SQ
        x = (yf + cdf * xt) / (1.0 + cdf)

    return x.reshape(B, C, H, W).astype(np.float32)
